# revision 25
# baseline (speedup 1.0000x reference)
"""Disentangled attention (fused common+personal QKV projections + MHA) on 8 TRN2 cores.

Strategy: data-parallel over batch N=8 (one batch element per NeuronCore, zero
communication). Host pre-sums W_c+W_p / b_c+b_p (exact), casts x/W to bf16, and
pre-transposes x so the device only sees x^T.

v3 pipeline (S=1024, D=512, H=8, hd=64), ACT-engine-centric:
  - ScalarE exp is the hard floor (8.4M exps ~ 55us @ 128 lanes * 1.2GHz), so
    the kernel keeps ACT saturated end-to-end: energies flow through TWO
    ping-ponged 3-bank PSUM slabs, each drained by ONE N<=1536 activation
    (PE writes slab B while ACT reads slab A; acts run back-to-back)
  - energy units are c-major (u = c*16 + 2j + h01): the first slab needs only
    the c0 projection quarters, and each pair's attn@V rides inside its OWN
    round (the post-exp tail is just the last c1 half-chains)
  - attn@V processes a HEAD PAIR per matmul slot via column tiling: head 2hp
    on PE columns 0-63, head 2hp+1 on columns 64-127, concurrently -- half the
    slots of the single-head [v|ones] formulation
  - softmax denominators come from dedicated 4-way column-tiled ones-matmul
    chains (4 heads per slot), batched per head-quad; they are transposed once
    per (quad, c) as a tiny [16, 512] xbar op
  - per-b weight tiles on 3 DMA queues, critical-first: the first projection
    k-chain starts as soon as ~1.25MB (xT + b0 weight blocks) lands
  - emission is slab-granular round-robin with explicitly gated filler slots
    (a filler after act n may only consume exps from acts <= n)
"""

import os
from contextlib import ExitStack

import numpy as np
import ml_dtypes

import concourse.bass as bass
import concourse.tile as tile
import concourse.mybir as mybir
from concourse import bacc
from concourse.bass_utils import run_bass_kernel_spmd

B, S, D, H, HD = 8, 1024, 512, 8, 64
P = 128
KB = D // P           # 4 contraction blocks
SB = S // P           # 8 sequence tiles
NU = 4 * SB           # 32 energy units per head pair
SU = 3                # units per PSUM slab (3 banks)
NS = (NU + SU - 1) // SU
BF16 = mybir.dt.bfloat16
F32 = mybir.dt.float32
SCALE = 1.0 / float(np.sqrt(D))

NPBF16 = ml_dtypes.bfloat16


def _unit(j, c, h01):
    return c * 16 + 2 * j + h01


def _bcast_ap(ap, parts):
    """Broadcast a [1, ...] AP across `parts` partitions (stride-0 partition dim)."""
    return bass.AP(tensor=ap.tensor, offset=ap.offset, ap=[[0, parts]] + list(ap.ap[1:]))


def emit_kernel(ctx: ExitStack, tc: tile.TileContext):
    nc = tc.nc

    xT_d = nc.dram_tensor("xT", [P, KB, S], BF16, kind="ExternalInput")
    wq_d = nc.dram_tensor("wq", [P, KB, KB, P], BF16, kind="ExternalInput")  # [p, k, b, col]
    wk_d = nc.dram_tensor("wk", [P, KB, KB, P], BF16, kind="ExternalInput")
    wv_d = nc.dram_tensor("wv", [P, KB, D], BF16, kind="ExternalInput")
    bq_d = nc.dram_tensor("bq", [P, KB], F32, kind="ExternalInput")
    bk_d = nc.dram_tensor("bk", [P, KB], F32, kind="ExternalInput")
    bv_d = nc.dram_tensor("bv", [1, D], F32, kind="ExternalInput")
    out_d = nc.dram_tensor("out", [S, D], BF16, kind="ExternalOutput")

    consts = ctx.enter_context(tc.tile_pool(name="consts", bufs=1))
    persist = ctx.enter_context(tc.tile_pool(name="persist", bufs=1))

    xTk = [persist.tile([P, S], BF16, tag=f"xT{k}", name=f"xT{k}") for k in range(KB)]
    wqb = [persist.tile([P, KB, P], BF16, tag=f"wq{b}", name=f"wq{b}") for b in range(KB)]
    wkb = [persist.tile([P, KB, P], BF16, tag=f"wk{b}", name=f"wk{b}") for b in range(KB)]
    wv_sb = persist.tile([P, KB, D], BF16, tag="wv", name="wv")
    bq_sb = consts.tile([P, KB], F32, tag="bq", name="bq")
    bk_sb = consts.tile([P, KB], F32, tag="bk", name="bk")
    bv_sb = consts.tile([P, D], F32, tag="bv", name="bv")

    # zt memset leads the gpsimd queue so HAM warmup can start immediately
    zt = consts.tile([P, 512], BF16, tag="zt", name="zt")
    nc.gpsimd.memset(zt[:], 0.0)

    # ---- loads on 3 queues, critical-first (xT + b0 weight blocks gate qk0)
    nc.sync.dma_start(out=xTk[0][:], in_=xT_d[:, 0, :])
    nc.scalar.dma_start(out=wqb[0][:], in_=wq_d[:, :, 0, :])
    nc.gpsimd.dma_start(out=wkb[0][:], in_=wk_d[:, :, 0, :])
    nc.sync.dma_start(out=xTk[1][:], in_=xT_d[:, 1, :])
    nc.scalar.dma_start(out=xTk[2][:], in_=xT_d[:, 2, :])
    nc.gpsimd.dma_start(out=xTk[3][:], in_=xT_d[:, 3, :])
    nc.sync.dma_start(out=bq_sb[:], in_=bq_d[:])
    nc.sync.dma_start(out=bk_sb[:], in_=bk_d[:])
    nc.scalar.dma_start(out=wqb[1][:], in_=wq_d[:, :, 1, :])
    nc.gpsimd.dma_start(out=wkb[1][:], in_=wk_d[:, :, 1, :])
    nc.scalar.dma_start(out=wqb[2][:], in_=wq_d[:, :, 2, :])
    nc.gpsimd.dma_start(out=wkb[2][:], in_=wk_d[:, :, 2, :])
    nc.scalar.dma_start(out=wqb[3][:], in_=wq_d[:, :, 3, :])
    nc.gpsimd.dma_start(out=wkb[3][:], in_=wk_d[:, :, 3, :])
    nc.sync.dma_start(out=wv_sb[:, 0:2, :], in_=wv_d[:, 0:2, :])
    nc.gpsimd.dma_start(out=wv_sb[:, 2:4, :], in_=wv_d[:, 2:4, :])
    nc.scalar.dma_start(out=bv_sb[:], in_=_bcast_ap(bv_d[:], P))

    qT_sb = [persist.tile([P, S], BF16, tag=f"qT{b}", name=f"qT{b}") for b in range(KB)]
    kT_sb = [persist.tile([P, S], BF16, tag=f"kT{b}", name=f"kT{b}") for b in range(KB)]
    v64_sb = [persist.tile([P, H, HD], BF16, tag=f"v64_{j}", name=f"v64_{j}") for j in range(SB)]

    # ones column for the 4-way denominator matmuls
    ones32 = consts.tile([P, 32], BF16, tag="ones32", name="ones32")
    nc.gpsimd.memset(ones32[:], 0.0)
    nc.gpsimd.memset(ones32[:, 0:1], 1.0)

    ptpool = ctx.enter_context(tc.tile_pool(name="ptpool", bufs=3))
    outTpool = ctx.enter_context(tc.tile_pool(name="outTpool", bufs=4))
    stagepool = ctx.enter_context(tc.tile_pool(name="stagepool", bufs=1))
    transpool = ctx.enter_context(tc.tile_pool(name="transpool", bufs=4))
    dnpool = ctx.enter_context(tc.tile_pool(name="dnpool", bufs=1))
    rpool = ctx.enter_context(tc.tile_pool(name="rpool", bufs=4))
    # PSUM: 2x 3-bank slabs (ping-pong) + 2 one-bank pp slots = 8 banks
    ppsum = ctx.enter_context(tc.tile_pool(name="ppsum", bufs=2, space="PSUM"))

    stage_sb = stagepool.tile([P, SB, H, HD], BF16, tag="stage", name="stage")
    outTp = [outTpool.tile([P, S], BF16, tag="outTp", name=f"outTp{hp}") for hp in range(4)]
    # denominators ride a full-width evac + standard [128,512] transpose;
    # head t's denominator lands in column 32*t of dtr
    dn = {(q, c): dnpool.tile([P, 512], BF16, tag=f"dn{q}{c}", name=f"dn{q}_{c}")
          for q in range(2) for c in range(2)}
    dtr = {(q, c): dnpool.tile([P, 4, P], BF16, tag=f"dtr{q}{c}", name=f"dtr{q}_{c}")
           for q in range(2) for c in range(2)}

    # ---- emission primitives -------------------------------------------
    def proj_qk_quarter(b, t, c):
        """one (dout-block, {q|k}, c-half): 4-matmul k-chain + DVE evac"""
        wb, b_sb, dst = ((wqb, bq_sb, qT_sb), (wkb, bk_sb, kT_sb))[t]
        ps = ppsum.tile([P, 512], F32, tag="pp", name=f"pp{b}_{t}_{c}")
        for k in range(KB):
            nc.tensor.matmul(
                ps[:],
                wb[b][:, k, :],
                xTk[k][:, c * 512:(c + 1) * 512],
                start=(k == 0), stop=(k == KB - 1),
            )
        nc.vector.tensor_scalar_add(
            out=dst[b][:, c * 512:(c + 1) * 512],
            in0=ps[:],
            scalar1=b_sb[:, b:b + 1],
        )

    def proj_v_block(j):
        pv = ppsum.tile([P, 512], F32, tag="pp", name=f"pv{j}")
        for k in range(KB):
            nc.tensor.matmul(
                pv[:],
                xTk[k][:, j * P:(j + 1) * P],
                wv_sb[:, k, :],
                start=(k == 0), stop=(k == KB - 1),
            )
        nc.vector.tensor_add(
            out=v64_sb[j][:],
            in0=pv[:].rearrange("p (h d) -> p h d", h=H),
            in1=bv_sb[:].rearrange("p (h d) -> p h d", h=H),
        )

    def energy_slab(hp, n, ptf):
        """matmuls for units 3n..3n+2 into a 3-bank slab, then ONE N<=1536 exp.
        Adjacent (h01=0,1) units run concurrently on the PE (row tiling)."""
        units = list(range(SU * n, min(SU * n + SU, NU)))
        slab = ppsum.tile([P, SU, 512], F32, tag="slab", name=f"slab{hp}_{n}", bufs=2)
        for i, u in enumerate(units):
            c, j, h01 = u // 16, (u % 16) // 2, u % 2
            rows = slice(h01 * 64, h01 * 64 + 64)
            nc.tensor.matmul(
                slab[:, i, :],
                kT_sb[hp][rows, j * P:(j + 1) * P],
                qT_sb[hp][rows, c * 512:(c + 1) * 512],
                start=True, stop=True,
                tile_position=(h01 * 64, 0),
            )
        nc.scalar.activation(
            out=ptf[:, units[0]:units[-1] + 1, :],
            in_=slab[:, 0:len(units), :],
            func=mybir.ActivationFunctionType.Exp,
            scale=SCALE,
        )

    def attn_pair_half(hp, c, half, box):
        """half of a column-tiled attn@V pair chain: both heads of the pair
        accumulate concurrently (cols 0-63 / 64-127). The second half
        evacuates outT^T and transposes it (normalization happens later,
        once the quad's denominators exist)."""
        if half == 0:
            box[0] = ppsum.tile([P, 512], F32, tag="pp", name=f"aop{hp}_{c}")
        ao = box[0]
        for j in range(half * 4, half * 4 + 4):
            for h01 in range(2):
                nc.tensor.matmul(
                    ao[h01 * 64:(h01 + 1) * 64, :],
                    v64_sb[j][:, 2 * hp + h01, :],
                    pts[hp][:, _unit(j, c, h01), :],
                    start=(j == 0), stop=(j == SB - 1),
                    tile_position=(0, h01 * 64),
                    skip_group_check=True,
                )
        if half == 1:
            sl = slice(c * 512, (c + 1) * 512)
            nc.vector.tensor_copy(out=outTp[hp][:, sl], in_=ao[:])
            trans = transpool.tile([P, 4, P], BF16, tag="trans", name=f"tr{hp}_{c}")
            nc.sync.dma_start_transpose(out=trans[:], in_=outTp[hp][:, sl])
            tr_tiles[(hp, c)] = trans

    def denom_half(q, c, half, box):
        """half of a 4-way column-tiled ones-matmul denominator chain for
        head-quad q (js 0-3 or 4-7); second half evacuates + transposes"""
        if half == 0:
            box[0] = ppsum.tile([P, 512], F32, tag="pp", name=f"dq{q}_{c}")
        dq = box[0]
        for j in range(half * 4, half * 4 + 4):
            for t in range(4):
                hp, h01 = 2 * q + t // 2, t % 2
                nc.tensor.matmul(
                    dq[32 * t:32 * t + 32, :],
                    ones32[:],
                    pts[hp][:, _unit(j, c, h01), :],
                    start=(j == 0), stop=(j == SB - 1),
                    tile_position=(0, 32 * t),
                    skip_group_check=True,
                )
        if half == 1:
            nc.vector.tensor_copy(out=dn[(q, c)][:], in_=dq[:])
            # post-act tail transposes ride the now-idle scalar queue
            teng = nc.scalar if (q, c) == (1, 1) else nc.sync
            teng.dma_start_transpose(out=dtr[(q, c)][:], in_=dn[(q, c)][:])

    def norm_pair(hp, c):
        """reciprocal of the transposed denominators + broadcast multiply of
        the transposed attn output into the staging buffer"""
        q = hp // 2
        trans = tr_tiles[(hp, c)]
        for h01 in range(2):
            h = 2 * hp + h01
            col = 32 * ((hp % 2) * 2 + h01)
            rc = rpool.tile([P, 4, 1], F32, tag="rc", name=f"rc{h}_{c}")
            nc.vector.reciprocal(out=rc[:], in_=dtr[(q, c)][:, :, col:col + 1])
            eng = nc.vector if hp == 3 else nc.gpsimd
            eng.tensor_mul(
                out=stage_sb[:, c * 4:(c + 1) * 4, h, :],
                in0=trans[:, :, h01 * 64:(h01 + 1) * 64],
                in1=rc[:].to_broadcast((P, 4, HD)),
            )

    out_v3 = out_d[:].rearrange("(j p) e -> p j e", p=P)

    def store(j0, j1, eng):
        eng.dma_start(
            out=out_v3[:, j0:j1, :],
            in_=stage_sb[:, j0:j1, :, :].rearrange("p j h d -> p j (h d)"),
        )

    # ---- prologue ------------------------------------------------------
    zp = ppsum.tile([P, 512], F32, tag="pp", name="warm")
    for w in range(10):
        nc.tensor.matmul(zp[:], zt[:, 0:P], zt[:], start=(w == 0), stop=(w == 9))

    # only the c0 quarters gate slab(0,0) (c-major unit order)
    proj_qk_quarter(0, 0, 0)
    proj_qk_quarter(0, 1, 0)

    pts = [None] * 4
    tr_tiles = {}
    boxes = {}

    def A(hp, c, half):
        key = (hp, c)
        if half == 0:
            boxes[key] = [None]
        return lambda: attn_pair_half(hp, c, half, boxes[key])

    def Q(b, t, c):
        return lambda: proj_qk_quarter(b, t, c)

    def V(j):
        return lambda: proj_v_block(j)

    def DN(q, c, half):
        key = ("dn", q, c)
        if half == 0:
            boxes[key] = [None]
        return lambda: denom_half(q, c, half, boxes[key])

    def NP(hp, c):
        return lambda: norm_pair(hp, c)

    def C(*fns):
        return lambda: [f() for f in fns]

    def ST(j0, j1, eng):
        return lambda: store(j0, j1, {"sync": nc.sync, "scalar": nc.scalar}[eng])

    # filler schedule: entry g of round hp runs after act g; it may only
    # consume exps from acts <= g of that round (gating) and everything
    # emitted earlier. None = no filler for that gap.
    fill = {
        0: [Q(0, 0, 1), Q(0, 1, 1), Q(1, 0, 0), Q(1, 0, 1), Q(1, 1, 0), Q(1, 1, 1),
            V(0), V(1), V(2), V(3), V(4)],
        1: [V(5), V(6), V(7), Q(2, 0, 0), Q(2, 0, 1), Q(2, 1, 0), Q(2, 1, 1),
            A(0, 0, 0), A(0, 0, 1), A(0, 1, 0), A(0, 1, 1)],
        2: [Q(3, 0, 0), Q(3, 0, 1), Q(3, 1, 0), Q(3, 1, 1),
            DN(0, 0, 0), DN(0, 0, 1), DN(0, 1, 0),
            C(DN(0, 1, 1), NP(0, 0), NP(0, 1)),
            A(1, 0, 0), A(1, 0, 1), A(1, 1, 0)],
        3: [C(A(1, 1, 1), NP(1, 0), NP(1, 1)),
            A(2, 0, 0), A(2, 0, 1), A(2, 1, 0), A(2, 1, 1),
            A(3, 0, 0), A(3, 0, 1), DN(1, 0, 0), DN(1, 0, 1),
            C(A(3, 1, 0), NP(2, 0), NP(3, 0), ST(0, 4, "sync")),
            DN(1, 1, 0),
            C(DN(1, 1, 1), A(3, 1, 1), NP(2, 1))],
    }

    for hp in range(4):
        pts[hp] = ptpool.tile([P, NU, 512], BF16, tag="pt", name=f"pt{hp}")
        fl = fill[hp]
        fi = 0
        for n in range(NS):
            energy_slab(hp, n, pts[hp])
            while fi < len(fl) and fi <= n:
                if fl[fi] is not None:
                    fl[fi]()
                fi += 1
        while fi < len(fl):
            if fl[fi] is not None:
                fl[fi]()
            fi += 1

    # ---- epilogue: last normalization + remaining stores ---------------
    norm_pair(3, 1)
    store(4, 6, nc.scalar)
    store(6, 8, nc.gpsimd)


_NC_CACHE = {}


def build_nc():
    if "nc" in _NC_CACHE:
        return _NC_CACHE["nc"]
    nc = bacc.Bacc("TRN2", target_bir_lowering=False, debug=False, num_devices=8)
    with tile.TileContext(nc) as tc:
        with ExitStack() as ctx:
            emit_kernel(ctx, tc)
    nc.compile()
    _NC_CACHE["nc"] = nc
    return nc


def host_prep(x, W_cq, b_cq, W_ck, b_ck, W_cv, b_cv, W_pq, b_pq, W_pk, b_pk, W_pv, b_pv):
    """Host-side sharding: exact f32 weight/bias fusion, bf16 casts, x transpose."""
    def blockw_bk(a, b2):
        # [din, dout] -> [p, k, b, col] (din = k*128+p, dout = b*128+col)
        w = (np.asarray(a, np.float32) + np.asarray(b2, np.float32)).astype(NPBF16)
        return np.ascontiguousarray(w.reshape(KB, P, KB, P).transpose(1, 0, 2, 3))

    def blockw_k(a, b2):
        w = (np.asarray(a, np.float32) + np.asarray(b2, np.float32)).astype(NPBF16)
        return np.ascontiguousarray(w.reshape(KB, P, D).transpose(1, 0, 2))

    wq = blockw_bk(W_cq, W_pq)
    wk = blockw_bk(W_ck, W_pk)
    wv = blockw_k(W_cv, W_pv)
    bq = (np.asarray(b_cq, np.float32) + np.asarray(b_pq, np.float32)).reshape(KB, P).T.copy()
    bk = (np.asarray(b_ck, np.float32) + np.asarray(b_pk, np.float32)).reshape(KB, P).T.copy()
    bv = (np.asarray(b_cv, np.float32) + np.asarray(b_pv, np.float32)).reshape(1, D).copy()
    x = np.asarray(x, np.float32)
    in_maps = []
    for n in range(B):
        xT = np.ascontiguousarray(
            x[n].T.astype(NPBF16).reshape(KB, P, S).transpose(1, 0, 2))
        in_maps.append({
            "xT": xT, "wq": wq, "wk": wk, "wv": wv,
            "bq": bq, "bk": bk, "bv": bv,
        })
    return in_maps


def kernel(**inputs) -> np.ndarray:
    in_maps = host_prep(**inputs)
    nc = build_nc()
    res = run_bass_kernel_spmd(
        nc, in_maps, core_ids=list(range(B)),
        trace=bool(int(os.environ.get("KERNEL_TRACE", "0"))),
    )
    _NC_CACHE["last_res"] = res
    if res.exec_time_ns is not None:
        print(f"HW exec time: {res.exec_time_ns} ns")
    out = np.stack([res.results[i]["out"] for i in range(B)], axis=0)
    return out.astype(np.float32)


# revision 26
# speedup vs baseline: 1.0173x; 1.0173x over previous
"""Disentangled attention (fused common+personal QKV projections + MHA) on 8 TRN2 cores.

Strategy: data-parallel over batch N=8 (one batch element per NeuronCore, zero
communication). Host pre-sums W_c+W_p / b_c+b_p (exact), casts x/W to bf16, and
pre-transposes x so the device only sees x^T.

v3 pipeline (S=1024, D=512, H=8, hd=64), ACT-engine-centric:
  - ScalarE exp is the hard floor (8.4M exps ~ 55us @ 128 lanes * 1.2GHz), so
    the kernel keeps ACT saturated end-to-end: energies flow through TWO
    ping-ponged 3-bank PSUM slabs, each drained by ONE N<=1536 activation
    (PE writes slab B while ACT reads slab A; acts run back-to-back)
  - energy units are c-major (u = c*16 + 2j + h01): the first slab needs only
    the c0 projection quarters, and each pair's attn@V rides inside its OWN
    round (the post-exp tail is just the last c1 half-chains)
  - attn@V processes a HEAD PAIR per matmul slot via column tiling: head 2hp
    on PE columns 0-63, head 2hp+1 on columns 64-127, concurrently -- half the
    slots of the single-head [v|ones] formulation
  - softmax denominators come from dedicated 4-way column-tiled ones-matmul
    chains (4 heads per slot), batched per head-quad; they are transposed once
    per (quad, c) as a tiny [16, 512] xbar op
  - per-b weight tiles on 3 DMA queues, critical-first: the first projection
    k-chain starts as soon as ~1.25MB (xT + b0 weight blocks) lands
  - emission is slab-granular round-robin with explicitly gated filler slots
    (a filler after act n may only consume exps from acts <= n)
"""

import os
from contextlib import ExitStack

import numpy as np
import ml_dtypes

import concourse.bass as bass
import concourse.tile as tile
import concourse.mybir as mybir
from concourse import bacc
from concourse.bass_utils import run_bass_kernel_spmd

B, S, D, H, HD = 8, 1024, 512, 8, 64
P = 128
KB = D // P           # 4 contraction blocks
SB = S // P           # 8 sequence tiles
NU = 4 * SB           # 32 energy units per head pair
SU = 3                # units per PSUM slab (3 banks)
NS = (NU + SU - 1) // SU
BF16 = mybir.dt.bfloat16
F32 = mybir.dt.float32
SCALE = 1.0 / float(np.sqrt(D))

NPBF16 = ml_dtypes.bfloat16


def _unit(j, c, h01):
    return c * 16 + 2 * j + h01


def _bcast_ap(ap, parts):
    """Broadcast a [1, ...] AP across `parts` partitions (stride-0 partition dim)."""
    return bass.AP(tensor=ap.tensor, offset=ap.offset, ap=[[0, parts]] + list(ap.ap[1:]))


def emit_kernel(ctx: ExitStack, tc: tile.TileContext):
    nc = tc.nc

    xT_d = nc.dram_tensor("xT", [P, KB, S], BF16, kind="ExternalInput")
    wq_d = nc.dram_tensor("wq", [P, KB, KB, P], BF16, kind="ExternalInput")  # [p, k, b, col]
    wk_d = nc.dram_tensor("wk", [P, KB, KB, P], BF16, kind="ExternalInput")
    wv_d = nc.dram_tensor("wv", [P, KB, D], BF16, kind="ExternalInput")
    bq_d = nc.dram_tensor("bq", [P, KB], F32, kind="ExternalInput")
    bk_d = nc.dram_tensor("bk", [P, KB], F32, kind="ExternalInput")
    bv_d = nc.dram_tensor("bv", [1, D], F32, kind="ExternalInput")
    out_d = nc.dram_tensor("out", [S, D], BF16, kind="ExternalOutput")

    consts = ctx.enter_context(tc.tile_pool(name="consts", bufs=1))
    persist = ctx.enter_context(tc.tile_pool(name="persist", bufs=1))

    xTk = [persist.tile([P, S], BF16, tag=f"xT{k}", name=f"xT{k}") for k in range(KB)]
    wqb = [persist.tile([P, KB, P], BF16, tag=f"wq{b}", name=f"wq{b}") for b in range(KB)]
    wkb = [persist.tile([P, KB, P], BF16, tag=f"wk{b}", name=f"wk{b}") for b in range(KB)]
    wv_sb = persist.tile([P, KB, D], BF16, tag="wv", name="wv")
    bq_sb = consts.tile([P, KB], F32, tag="bq", name="bq")
    bk_sb = consts.tile([P, KB], F32, tag="bk", name="bk")
    bv_sb = consts.tile([P, D], F32, tag="bv", name="bv")

    # zt memset leads the gpsimd queue so HAM warmup can start immediately
    zt = consts.tile([P, 512], BF16, tag="zt", name="zt")
    nc.gpsimd.memset(zt[:], 0.0)

    # ---- loads on 3 queues, critical-first (xT + b0 weight blocks gate qk0)
    nc.sync.dma_start(out=xTk[0][:], in_=xT_d[:, 0, :])
    nc.scalar.dma_start(out=wqb[0][:], in_=wq_d[:, :, 0, :])
    nc.gpsimd.dma_start(out=wkb[0][:], in_=wk_d[:, :, 0, :])
    nc.sync.dma_start(out=xTk[1][:], in_=xT_d[:, 1, :])
    nc.scalar.dma_start(out=xTk[2][:], in_=xT_d[:, 2, :])
    nc.gpsimd.dma_start(out=xTk[3][:], in_=xT_d[:, 3, :])
    nc.sync.dma_start(out=bq_sb[:], in_=bq_d[:])
    nc.sync.dma_start(out=bk_sb[:], in_=bk_d[:])
    nc.scalar.dma_start(out=wqb[1][:], in_=wq_d[:, :, 1, :])
    nc.gpsimd.dma_start(out=wkb[1][:], in_=wk_d[:, :, 1, :])
    nc.scalar.dma_start(out=wqb[2][:], in_=wq_d[:, :, 2, :])
    nc.gpsimd.dma_start(out=wkb[2][:], in_=wk_d[:, :, 2, :])
    nc.scalar.dma_start(out=wqb[3][:], in_=wq_d[:, :, 3, :])
    nc.gpsimd.dma_start(out=wkb[3][:], in_=wk_d[:, :, 3, :])
    nc.sync.dma_start(out=wv_sb[:, 0:2, :], in_=wv_d[:, 0:2, :])
    nc.gpsimd.dma_start(out=wv_sb[:, 2:4, :], in_=wv_d[:, 2:4, :])
    nc.scalar.dma_start(out=bv_sb[:], in_=_bcast_ap(bv_d[:], P))

    qT_sb = [persist.tile([P, S], BF16, tag=f"qT{b}", name=f"qT{b}") for b in range(KB)]
    kT_sb = [persist.tile([P, S], BF16, tag=f"kT{b}", name=f"kT{b}") for b in range(KB)]
    v64_sb = [persist.tile([P, H, HD], BF16, tag=f"v64_{j}", name=f"v64_{j}") for j in range(SB)]

    # ones column for the 4-way denominator matmuls
    ones32 = consts.tile([P, 32], BF16, tag="ones32", name="ones32")
    nc.gpsimd.memset(ones32[:], 0.0)
    nc.gpsimd.memset(ones32[:, 0:1], 1.0)

    ptpool = ctx.enter_context(tc.tile_pool(name="ptpool", bufs=3))
    outTpool = ctx.enter_context(tc.tile_pool(name="outTpool", bufs=4))
    stagepool = ctx.enter_context(tc.tile_pool(name="stagepool", bufs=1))
    transpool = ctx.enter_context(tc.tile_pool(name="transpool", bufs=4))
    dnpool = ctx.enter_context(tc.tile_pool(name="dnpool", bufs=1))
    rpool = ctx.enter_context(tc.tile_pool(name="rpool", bufs=4))
    # PSUM: 2x 3-bank slabs (ping-pong) + 2 one-bank pp slots = 8 banks
    ppsum = ctx.enter_context(tc.tile_pool(name="ppsum", bufs=2, space="PSUM"))

    stage_sb = stagepool.tile([P, SB, H, HD], BF16, tag="stage", name="stage")
    outTp = [outTpool.tile([P, S], BF16, tag="outTp", name=f"outTp{hp}") for hp in range(4)]
    # denominators ride a full-width evac + standard [128,512] transpose;
    # head t's denominator lands in column 32*t of dtr
    dn = {(q, c): dnpool.tile([P, 512], BF16, tag=f"dn{q}{c}", name=f"dn{q}_{c}")
          for q in range(2) for c in range(2)}
    dtr = {(q, c): dnpool.tile([P, 4, P], BF16, tag=f"dtr{q}{c}", name=f"dtr{q}_{c}")
           for q in range(2) for c in range(2)}

    # ---- emission primitives -------------------------------------------
    def proj_qk_quarter(b, t, c):
        """one (dout-block, {q|k}, c-half): 4-matmul k-chain + DVE evac"""
        wb, b_sb, dst = ((wqb, bq_sb, qT_sb), (wkb, bk_sb, kT_sb))[t]
        ps = ppsum.tile([P, 512], F32, tag="pp", name=f"pp{b}_{t}_{c}")
        for k in range(KB):
            nc.tensor.matmul(
                ps[:],
                wb[b][:, k, :],
                xTk[k][:, c * 512:(c + 1) * 512],
                start=(k == 0), stop=(k == KB - 1),
            )
        nc.vector.tensor_scalar_add(
            out=dst[b][:, c * 512:(c + 1) * 512],
            in0=ps[:],
            scalar1=b_sb[:, b:b + 1],
        )

    def proj_v_block(j):
        pv = ppsum.tile([P, 512], F32, tag="pp", name=f"pv{j}")
        for k in range(KB):
            nc.tensor.matmul(
                pv[:],
                xTk[k][:, j * P:(j + 1) * P],
                wv_sb[:, k, :],
                start=(k == 0), stop=(k == KB - 1),
            )
        nc.vector.tensor_add(
            out=v64_sb[j][:],
            in0=pv[:].rearrange("p (h d) -> p h d", h=H),
            in1=bv_sb[:].rearrange("p (h d) -> p h d", h=H),
        )

    def energy_slab(hp, n, ptf):
        """matmuls for units 3n..3n+2 into a 3-bank slab, then ONE N<=1536 exp.
        Adjacent (h01=0,1) units run concurrently on the PE (row tiling)."""
        units = list(range(SU * n, min(SU * n + SU, NU)))
        slab = ppsum.tile([P, SU, 512], F32, tag="slab", name=f"slab{hp}_{n}", bufs=2)
        for i, u in enumerate(units):
            c, j, h01 = u // 16, (u % 16) // 2, u % 2
            rows = slice(h01 * 64, h01 * 64 + 64)
            nc.tensor.matmul(
                slab[:, i, :],
                kT_sb[hp][rows, j * P:(j + 1) * P],
                qT_sb[hp][rows, c * 512:(c + 1) * 512],
                start=True, stop=True,
                tile_position=(h01 * 64, 0),
            )
        nc.scalar.activation(
            out=ptf[:, units[0]:units[-1] + 1, :],
            in_=slab[:, 0:len(units), :],
            func=mybir.ActivationFunctionType.Exp,
            scale=SCALE,
        )

    def attn_pair_half(hp, c, half, box):
        """half of a column-tiled attn@V pair chain: both heads of the pair
        accumulate concurrently (cols 0-63 / 64-127). The second half
        evacuates outT^T and transposes it (normalization happens later,
        once the quad's denominators exist)."""
        if half == 0:
            box[0] = ppsum.tile([P, 512], F32, tag="pp", name=f"aop{hp}_{c}")
        ao = box[0]
        for j in range(half * 4, half * 4 + 4):
            for h01 in range(2):
                nc.tensor.matmul(
                    ao[h01 * 64:(h01 + 1) * 64, :],
                    v64_sb[j][:, 2 * hp + h01, :],
                    pts[hp][:, _unit(j, c, h01), :],
                    start=(j == 0), stop=(j == SB - 1),
                    tile_position=(0, h01 * 64),
                    skip_group_check=True,
                )
        if half == 1:
            sl = slice(c * 512, (c + 1) * 512)
            nc.vector.tensor_copy(out=outTp[hp][:, sl], in_=ao[:])
            trans = transpool.tile([P, 4, P], BF16, tag="trans", name=f"tr{hp}_{c}")
            nc.sync.dma_start_transpose(out=trans[:], in_=outTp[hp][:, sl])
            tr_tiles[(hp, c)] = trans

    def denom_half(q, c, half, box):
        """half of a 4-way column-tiled ones-matmul denominator chain for
        head-quad q (js 0-3 or 4-7); second half evacuates + transposes"""
        if half == 0:
            box[0] = ppsum.tile([P, 512], F32, tag="pp", name=f"dq{q}_{c}")
        dq = box[0]
        for j in range(half * 4, half * 4 + 4):
            for t in range(4):
                hp, h01 = 2 * q + t // 2, t % 2
                nc.tensor.matmul(
                    dq[32 * t:32 * t + 32, :],
                    ones32[:],
                    pts[hp][:, _unit(j, c, h01), :],
                    start=(j == 0), stop=(j == SB - 1),
                    tile_position=(0, 32 * t),
                    skip_group_check=True,
                )
        if half == 1:
            # the final quad's evac rides the post-acts-idle ScalarE so it
            # runs concurrently with the DVE CAST of the last attn pair
            if (q, c) == (1, 1):
                nc.scalar.copy(out=dn[(q, c)][:], in_=dq[:])
            else:
                nc.vector.tensor_copy(out=dn[(q, c)][:], in_=dq[:])
            teng = nc.scalar if (q, c) == (1, 1) else nc.sync
            teng.dma_start_transpose(out=dtr[(q, c)][:], in_=dn[(q, c)][:])

    def norm_pair(hp, c):
        """reciprocal of the transposed denominators + broadcast multiply of
        the transposed attn output into the staging buffer"""
        q = hp // 2
        trans = tr_tiles[(hp, c)]
        for h01 in range(2):
            h = 2 * hp + h01
            col = 32 * ((hp % 2) * 2 + h01)
            rc = rpool.tile([P, 4, 1], F32, tag="rc", name=f"rc{h}_{c}")
            nc.vector.reciprocal(out=rc[:], in_=dtr[(q, c)][:, :, col:col + 1])
            eng = nc.vector if hp == 3 else nc.gpsimd
            eng.tensor_mul(
                out=stage_sb[:, c * 4:(c + 1) * 4, h, :],
                in0=trans[:, :, h01 * 64:(h01 + 1) * 64],
                in1=rc[:].to_broadcast((P, 4, HD)),
            )

    out_v3 = out_d[:].rearrange("(j p) e -> p j e", p=P)

    def store(j0, j1, eng):
        eng.dma_start(
            out=out_v3[:, j0:j1, :],
            in_=stage_sb[:, j0:j1, :, :].rearrange("p j h d -> p j (h d)"),
        )

    # ---- prologue ------------------------------------------------------
    zp = ppsum.tile([P, 512], F32, tag="pp", name="warm")
    for w in range(10):
        nc.tensor.matmul(zp[:], zt[:, 0:P], zt[:], start=(w == 0), stop=(w == 9))

    # only the c0 quarters gate slab(0,0) (c-major unit order)
    proj_qk_quarter(0, 0, 0)
    proj_qk_quarter(0, 1, 0)

    pts = [None] * 4
    tr_tiles = {}
    boxes = {}

    def A(hp, c, half):
        key = (hp, c)
        if half == 0:
            boxes[key] = [None]
        return lambda: attn_pair_half(hp, c, half, boxes[key])

    def Q(b, t, c):
        return lambda: proj_qk_quarter(b, t, c)

    def V(j):
        return lambda: proj_v_block(j)

    def DN(q, c, half):
        key = ("dn", q, c)
        if half == 0:
            boxes[key] = [None]
        return lambda: denom_half(q, c, half, boxes[key])

    def NP(hp, c):
        return lambda: norm_pair(hp, c)

    def C(*fns):
        return lambda: [f() for f in fns]

    def ST(j0, j1, eng):
        return lambda: store(j0, j1, {"sync": nc.sync, "scalar": nc.scalar}[eng])

    # filler schedule: entry g of round hp runs after act g; it may only
    # consume exps from acts <= g of that round (gating) and everything
    # emitted earlier. None = no filler for that gap.
    fill = {
        0: [Q(0, 0, 1), Q(0, 1, 1), Q(1, 0, 0), Q(1, 0, 1), Q(1, 1, 0), Q(1, 1, 1),
            V(0), V(1), V(2), V(3), V(4)],
        1: [V(5), V(6), V(7), Q(2, 0, 0), Q(2, 0, 1), Q(2, 1, 0), Q(2, 1, 1),
            A(0, 0, 0), A(0, 0, 1), A(0, 1, 0), A(0, 1, 1)],
        2: [Q(3, 0, 0), Q(3, 0, 1), Q(3, 1, 0), Q(3, 1, 1),
            DN(0, 0, 0), DN(0, 0, 1), DN(0, 1, 0),
            C(DN(0, 1, 1), NP(0, 0), NP(0, 1)),
            A(1, 0, 0), A(1, 0, 1), A(1, 1, 0)],
        3: [C(A(1, 1, 1), NP(1, 0), NP(1, 1)),
            A(2, 0, 0), A(2, 0, 1), A(2, 1, 0), A(2, 1, 1),
            A(3, 0, 0), A(3, 0, 1), DN(1, 0, 0), DN(1, 0, 1),
            C(A(3, 1, 0), NP(2, 0), NP(3, 0), ST(0, 4, "sync")),
            DN(1, 1, 0),
            C(DN(1, 1, 1), A(3, 1, 1), NP(2, 1))],
    }

    for hp in range(4):
        pts[hp] = ptpool.tile([P, NU, 512], BF16, tag="pt", name=f"pt{hp}")
        fl = fill[hp]
        fi = 0
        for n in range(NS):
            energy_slab(hp, n, pts[hp])
            while fi < len(fl) and fi <= n:
                if fl[fi] is not None:
                    fl[fi]()
                fi += 1
        while fi < len(fl):
            if fl[fi] is not None:
                fl[fi]()
            fi += 1

    # ---- epilogue: last normalization + remaining stores ---------------
    norm_pair(3, 1)
    store(4, 6, nc.scalar)
    store(6, 8, nc.gpsimd)


_NC_CACHE = {}


def build_nc():
    if "nc" in _NC_CACHE:
        return _NC_CACHE["nc"]
    nc = bacc.Bacc("TRN2", target_bir_lowering=False, debug=False, num_devices=8)
    with tile.TileContext(nc) as tc:
        with ExitStack() as ctx:
            emit_kernel(ctx, tc)
    nc.compile()
    _NC_CACHE["nc"] = nc
    return nc


def host_prep(x, W_cq, b_cq, W_ck, b_ck, W_cv, b_cv, W_pq, b_pq, W_pk, b_pk, W_pv, b_pv):
    """Host-side sharding: exact f32 weight/bias fusion, bf16 casts, x transpose."""
    def blockw_bk(a, b2):
        # [din, dout] -> [p, k, b, col] (din = k*128+p, dout = b*128+col)
        w = (np.asarray(a, np.float32) + np.asarray(b2, np.float32)).astype(NPBF16)
        return np.ascontiguousarray(w.reshape(KB, P, KB, P).transpose(1, 0, 2, 3))

    def blockw_k(a, b2):
        w = (np.asarray(a, np.float32) + np.asarray(b2, np.float32)).astype(NPBF16)
        return np.ascontiguousarray(w.reshape(KB, P, D).transpose(1, 0, 2))

    wq = blockw_bk(W_cq, W_pq)
    wk = blockw_bk(W_ck, W_pk)
    wv = blockw_k(W_cv, W_pv)
    bq = (np.asarray(b_cq, np.float32) + np.asarray(b_pq, np.float32)).reshape(KB, P).T.copy()
    bk = (np.asarray(b_ck, np.float32) + np.asarray(b_pk, np.float32)).reshape(KB, P).T.copy()
    bv = (np.asarray(b_cv, np.float32) + np.asarray(b_pv, np.float32)).reshape(1, D).copy()
    x = np.asarray(x, np.float32)
    in_maps = []
    for n in range(B):
        xT = np.ascontiguousarray(
            x[n].T.astype(NPBF16).reshape(KB, P, S).transpose(1, 0, 2))
        in_maps.append({
            "xT": xT, "wq": wq, "wk": wk, "wv": wv,
            "bq": bq, "bk": bk, "bv": bv,
        })
    return in_maps


def kernel(**inputs) -> np.ndarray:
    in_maps = host_prep(**inputs)
    nc = build_nc()
    res = run_bass_kernel_spmd(
        nc, in_maps, core_ids=list(range(B)),
        trace=bool(int(os.environ.get("KERNEL_TRACE", "0"))),
    )
    _NC_CACHE["last_res"] = res
    if res.exec_time_ns is not None:
        print(f"HW exec time: {res.exec_time_ns} ns")
    out = np.stack([res.results[i]["out"] for i in range(B)], axis=0)
    return out.astype(np.float32)
